# revision 2
# baseline (speedup 1.0000x reference)
"""Sinkhorn optimal-transport transport-plan kernel for 8 Trainium2 NeuronCores.

Math (matches the reference):
    cost = sq_m[i] + sq_n[j] - 2 Hm@Hn^T  (the reference's max(.,0) clamp is a
    no-op up to fp rounding since cost = |hm_i - hn_j|^2 >= 0);
    K = exp(-cost/eps);  20x: u <- mu/(K @ (nu/(K^T @ u)));
    v = nu/(K^T u);  P = diag(u) K diag(v).

Change of variables: with X = exp(2 G/eps) (G = Hm@Hn^T), em = exp(-sq_m/eps),
en = exp(-sq_n/eps), K = diag(em) X diag(en).  Substituting u~ = em*u:
    u~ <- mu/(X (nu/(X^T u~))),  u~0 = em,
    v~ = nu/(X^T u~_20),  P = diag(u~) X diag(v~)
so en/em vanish from the loop and the plan entirely (em only seeds u~0).

Distribution: COLUMN shard - core c owns j in [c*1024, (c+1)*1024).  Pass A
(w~ = X^T u~, contraction over all 8192 i) is local-complete per core: no
collective.  Pass B (y~ = X x~) produces full-length-N partials -> one fp32
AllReduce of 32 KB per iteration, split in i-halves so each AR overlaps the
other half's matmuls / the next pass A.

Residency: X is quantized to fp8e4 (values in ~[0.36, 2.9]) and kept in SBUF
in BOTH orientations - X8 row-major [i-part, j] for pass A and XT8 col-major
[j-part, i] for pass B - 64 KB/partition each, so the loop runs with ZERO HBM
traffic.  Matvec moving operands (u~, x~) stay fp16 (fp8 vectors break the
2e-2 gate; fp8 X + fp16 vectors simulate to ~2.3e-3 absmax-rel).  The final
plan recomputes X in one fused pass: psum = G + (eps/2) ln v~_j (delta-row
matmul trick), then ACT exp(scale=2/eps, bias=ln u~_i) emits P directly.

kernel(H_m, H_n) takes the full inputs and returns the full (N, N) fp32 plan.
"""

import sys

for _p in ("/opt/trn_rl_repo", "/root/.axon_site", "/root/.axon_site/_ro/pypackages"):
    if _p not in sys.path:
        sys.path.append(_p)

import numpy as np

import concourse.bass as bass
import concourse.mybir as mybir
import concourse.tile as tile
from concourse.masks import make_identity

F32 = mybir.dt.float32
F16 = mybir.dt.float16
F8 = mybir.dt.float8e4
Exp = mybir.ActivationFunctionType.Exp
Log = mybir.ActivationFunctionType.Ln

EPS = 0.05
ITERS = 20
SU = 2.0           # u~ fp16 carry scale
SXC = float(2**24)  # x~ fp16 carry scale (x~ ~ 4e-8 is below fp16 normals)

MAX_WAITS = 1  # walrus codegen allows only one attached sync wait per inst


def _split_excess_waits(nc, maxw=MAX_WAITS):
    """Split excess per-instruction sync waits onto same-engine NoOps (engine
    queues execute in program order, so semantics are identical)."""
    for bb in nc.main_func.blocks:
        new = []
        for ins in bb.instructions:
            si = ins.sync_info
            if si is not None and len(si.on_wait) > maxw:
                waits = list(si.on_wait)
                excess, keep = waits[:-maxw], waits[-maxw:]
                for i in range(0, len(excess), maxw):
                    nop = mybir.InstNoOp(
                        name=nc.get_next_instruction_name(),
                        engine=ins.engine,
                        bass_nofuse=True,
                        sync_info=mybir.SyncInfo(
                            on_wait=excess[i : i + maxw], on_update=[]
                        ),
                    )
                    new.append(nop)
                ins.sync_info = mybir.SyncInfo(
                    on_wait=keep, on_update=list(si.on_update)
                )
            new.append(ins)
        bb.instructions = new


def build_nc(N=8192, D=128, ncores=8, split_waits=True, iters=ITERS,
             collective=True):
    assert D == 128 and N % (ncores * 128) == 0
    R = N // ncores   # local columns (j) per core
    CL = R // 128     # local j-chunks of 128      (8)
    S = N // 128      # i-stripes of 128           (64)
    HS = S // 2       # half the i-stripes         (32)
    P = 128

    nc = bass.Bass(num_devices=ncores)
    hmT = nc.declare_dram_parameter("hmT", [D, N], F32, isOutput=False)
    hnT = nc.declare_dram_parameter("hnT", [D, R], F32, isOutput=False)
    # transposed output shard P^T[j_local, i]: 32KB-contiguous DRAM rows
    # make the final-store DMAs run at full HBM write rate
    out = nc.declare_dram_parameter("out", [R, N], F32, isOutput=True)

    with tile.TileContext(nc) as tc:
        with (
            tc.tile_pool(name="persist", bufs=1) as sb,
            tc.tile_pool(name="dram", bufs=1, space="DRAM") as dram,
        ):
            # ---- persistent state ----
            x8 = sb.tile([P, S * R], F8, name="x8")      # X row-major, 64KB/p
            xt8 = sb.tile([P, CL * N], F8, name="xt8")   # X col-major, 64KB/p
            # f16 copies of the inputs: 1-cycle/row PE matmuls (fp32 pays
            # a 4x row penalty) and FWL weight loads; ~5e-4 impact on X
            hmT16 = sb.tile([P, N], F16, name="hmT16")    # 16KB/p
            hnT16 = sb.tile([P, R], F16, name="hnT16")    # 2KB/p
            u_sb = sb.tile([P, S], F16, name="u_sb")      # u~ * SU
            uf32 = sb.tile([P, S], F32, name="uf32")      # u~ (final exact)
            x_sb = sb.tile([P, CL], F16, name="x_sb")     # x~ * SXC
            wrec = sb.tile([P, CL], F32, name="wrec")     # 1/w~sb of last pass A
            ones_col = sb.tile([P, 1], F16, name="ones_col")
            ident = sb.tile([P, P], F32, name="ident")
            delta0 = sb.tile([P, P], F16, name="delta0")  # row 0 ones, else 0
            lnu_bc = sb.tile([P, N], F16, name="lnu_bc")  # row 0: (eps/2)ln u~

            # chunked (so build matmuls start on the first chunk) and
            # cast f32->f16 in flight (SWDGE)
            nc.gpsimd.dma_start(out=hnT16, in_=hnT[:, :])
            for q in range(0, N, 2048):
                nc.gpsimd.dma_start(out=hmT16[:, q : q + 2048],
                                    in_=hmT[:, q : q + 2048])
            nc.vector.memset(ones_col, 1.0)
            make_identity(nc, ident)
            nc.vector.memset(delta0, 0.0)
            nc.vector.memset(delta0[0:1, :], 1.0)
            nc.vector.memset(lnu_bc, 0.0)

            # ================= build: u~0 = em, X8, XT8 =================
            # single pool scope so the sq/em phase interleaves with the
            # X8/XT8 granule pipeline instead of serializing before it
            with (
                tc.tile_pool(name="bld_sb", bufs=2) as bs,
                tc.tile_pool(name="bld_ps", bufs=1, space="PSUM") as bp,
            ):
                # sq_m: per 512-chunk square then ones-matvec (part. red.)
                ps_sq = bp.tile([P, 512], F32, name="ps_sq", tag="sq")
                for q in range(0, N, 512):
                    sq_g = bs.tile([P, 512], F16, name=f"sq{q}", tag="sq_g")
                    nc.vector.tensor_mul(
                        sq_g, hmT16[:, q : q + 512], hmT16[:, q : q + 512]
                    )
                    for k in range(4):
                        s = q // 128 + k
                        nc.tensor.matmul(
                            out=ps_sq[:, s : s + 1],
                            lhsT=sq_g[:, k * P : (k + 1) * P],
                            rhs=ones_col, start=True, stop=True,
                        )
                # u~0 = em = exp(-sq_m/eps), both f16 (*SU) and f32
                nc.scalar.activation(uf32, ps_sq[:, :S], Exp,
                                     scale=-1.0 / EPS)
                nc.vector.tensor_scalar_mul(u_sb, uf32, SU)

                # X8: stripe s holds X[s*128+p, j] at [p, s*R + j]
                for s in range(S):
                    gx = bp.tile([P, R], F32, name=f"gx{s}", tag="bg", bufs=3)
                    for k in range(0, R, 512):
                        nc.tensor.matmul(
                            out=gx[:, k : k + 512],
                            lhsT=hmT16[:, s * P : (s + 1) * P],
                            rhs=hnT16[:, k : k + 512],
                            start=True, stop=True,
                        )
                    nc.scalar.activation(
                        x8[:, s * R : (s + 1) * R], gx, Exp, scale=2.0 / EPS
                    )
                # XT8: chunk c holds X[i, c*128+p] at [p, c*N + i]
                for c in range(CL):
                    for tg in range(0, N, R):
                        gt = bp.tile([P, R], F32, name=f"gt{c}_{tg}",
                                     tag="bg", bufs=3)
                        for k in range(0, R, 512):
                            nc.tensor.matmul(
                                out=gt[:, k : k + 512],
                                lhsT=hnT16[:, c * P : (c + 1) * P],
                                rhs=hmT16[:, tg + k : tg + k + 512],
                                start=True, stop=True,
                            )
                        nc.scalar.activation(
                            xt8[:, c * N + tg : c * N + tg + R], gt, Exp,
                            scale=2.0 / EPS,
                        )

            # ======================= Sinkhorn loop =======================
            tc.strict_bb_all_engine_barrier()
            with (
                tc.tile_pool(name="loop_sb", bufs=2) as lp,
                tc.tile_pool(name="loop_ps", bufs=4, space="PSUM") as lpp,
                tc.tile_pool(name="lp_ps2", bufs=2, space="PSUM") as lp2,
            ):
                pending = None  # yf tiles of the in-flight ARs (prev iter)

                def fold_u(h):
                    """u~ update for AR half h, emitted just-in-time so the
                    AR-dependent DVE ops never head-of-line-block pass A."""
                    rec = lp.tile([P, HS], F32, name=f"rec{it}_{h}",
                                  tag=f"rec{h}")
                    nc.vector.reciprocal(rec, pending[h])
                    sl = slice(h * HS, (h + 1) * HS)
                    # u~ = mu/y~ : uf32 = rec*SXC/N ; u_sb = uf32*SU
                    nc.vector.tensor_scalar_mul(uf32[:, sl], rec, SXC / N)
                    nc.vector.tensor_scalar_mul(u_sb[:, sl], rec,
                                                SU * SXC / N)

                for it in range(iters + 1):
                    # ---- pass A: w~[c*128+p] = sum_s X8(s,c)^T u~_s ----
                    # stripe-group-outer (8 stripes per psum group); group
                    # partials accumulate into SBUF on DVE.  Full-bank psum
                    # tiles keep PE writes and DVE reads in different banks
                    # (bank collisions are fatal).  Each u~ half is folded in
                    # right before the first group that consumes it.
                    w_acc = lp.tile([P, CL], F32, name=f"wacc{it}", tag="wacc")
                    for g in range(S // 8):
                        if g == 0 and pending is not None:
                            fold_u(0)
                        if g == 4 and pending is not None:
                            fold_u(1)
                            pending = None
                        pa = lpp.tile([P, 512], F32, name=f"pa{it}_{g}",
                                      tag="pa")
                        for c in range(CL):
                            for k in range(8):
                                s = g * 8 + k
                                nc.tensor.matmul(
                                    out=pa[:, c : c + 1],
                                    lhsT=x8[
                                        :, s * R + c * P : s * R + (c + 1) * P
                                    ],
                                    rhs=u_sb[:, s : s + 1],
                                    start=(k == 0), stop=(k == 7),
                                )
                        if g == 0:
                            nc.vector.tensor_copy(w_acc, pa[:, :CL])
                        else:
                            nc.vector.tensor_add(w_acc, w_acc, pa[:, :CL])
                    nc.vector.reciprocal(wrec, w_acc)
                    if it == iters:
                        break
                    # x~ * SXC = (1/w~sb) * SU*SXC/N
                    nc.vector.tensor_scalar_mul(x_sb, wrec, SU * SXC / N)

                    # ---- pass B: y~[t*128+p] partial = sum_c XT8(c,t) x~_c --
                    # then AllReduce in i-halves overlapped with matmuls.
                    pending = [None, None]
                    for h in range(2):
                        # full-bank tile so the two halves never share a bank
                        psy = lp2.tile([P, 512], F32, name=f"psy{it}_{h}",
                                       tag=f"psy{h}")
                        for t in range(HS):
                            ts = h * HS + t
                            for c in range(CL):
                                nc.tensor.matmul(
                                    out=psy[:, t : t + 1],
                                    lhsT=xt8[
                                        :, c * N + ts * P : c * N + (ts + 1) * P
                                    ],
                                    rhs=x_sb[:, c : c + 1],
                                    start=(c == 0), stop=(c == CL - 1),
                                )
                        # Evacuation/bounce engines are split per half so the
                        # h0 AR trigger chain (ACT queue) is never blocked
                        # behind h1 work or AR-completion waits.
                        y_sb = lp.tile([P, HS], F32, name=f"y{it}_{h}",
                                       tag=f"y{h}")
                        if h == 0:
                            nc.scalar.copy(y_sb, psy[:, :HS])
                        else:
                            nc.vector.tensor_copy(y_sb, psy[:, :HS])
                        y_in = dram.tile([P, HS], F32, name=f"yin{it}_{h}",
                                         tag=f"yin{h}", bufs=2)
                        y_out = dram.tile(
                            [P, HS], F32, name=f"yout{it}_{h}", tag=f"yout{h}",
                            bufs=2, addr_space="Shared",
                        )
                        eng = nc.scalar if h == 0 else nc.sync
                        eng.dma_start(out=y_in, in_=y_sb)
                        if collective:
                            nc.gpsimd.collective_compute(
                                "AllReduce", mybir.AluOpType.add,
                                replica_groups=[list(range(ncores))],
                                ins=[y_in.opt()], outs=[y_out.opt()],
                            )
                        else:
                            eng.dma_start(out=y_out, in_=y_in)
                        yf = lp.tile([P, HS], F32, name=f"yf{it}_{h}",
                                     tag=f"yf{h}")
                        eng.dma_start(out=yf, in_=y_out)
                        pending[h] = yf

            # ========= v~, then P^T[j,i] = exp(2G/eps + lnu + lnv) =========
            tc.strict_bb_all_engine_barrier()
            with tc.tile_pool(name="fin_pre", bufs=1, space="PSUM") as fq:
                with tc.tile_pool(name="fin_sb0", bufs=1) as f0:
                    # ln v~ per-partition (j local): ACT bias of the P^T pass
                    vr = f0.tile([P, CL], F32, name="vr")
                    # v~ = nu/w~ = wrec * SU/N
                    nc.vector.tensor_scalar_mul(vr, wrec, SU / N)
                    lnv = sb.tile([P, CL], F32, name="lnv")
                    nc.scalar.activation(lnv, vr, Log)
                    # (eps/2) ln u~ as a [1, N] f16 row for the delta-matmul
                    lnu = f0.tile([P, S], F32, name="lnu")
                    nc.scalar.activation(lnu, uf32, Log)
                    nc.vector.tensor_scalar_mul(lnu, lnu, EPS / 2.0)
                    tps = fq.tile([S, P], F32, name="tps")
                    nc.tensor.transpose(tps, lnu, ident)
                    t8 = f0.tile([S, P], F16, name="t8")
                    nc.vector.tensor_copy(t8, tps)
                    lnud = dram.tile([S, P], F16, name="lnud")
                    nc.sync.dma_start(out=lnud, in_=t8)
                    nc.sync.dma_start(
                        out=lnu_bc[0:1, :],
                        in_=lnud.rearrange("s p -> (s p)")[None, :],
                    )

            with (
                tc.tile_pool(name="fin_sb", bufs=3) as fp,
                tc.tile_pool(name="fin_ps", bufs=2, space="PSUM") as fpp,
            ):
                # P^T granules [128 j, 2048 i]; DRAM rows are 32KB contiguous
                for c in range(CL):
                    for tg in range(0, N, 2048):
                        gq = fpp.tile([P, 2048], F32, name=f"fg{c}_{tg}",
                                      tag="fg")
                        for k in range(0, 2048, 512):
                            nc.tensor.matmul(
                                out=gq[:, k : k + 512],
                                lhsT=hnT16[:, c * P : (c + 1) * P],
                                rhs=hmT16[:, tg + k : tg + k + 512],
                                start=True, stop=False,
                            )
                            nc.tensor.matmul(
                                out=gq[:, k : k + 512],
                                lhsT=delta0,
                                rhs=lnu_bc[:, tg + k : tg + k + 512],
                                start=False, stop=True,
                            )
                        pf = fp.tile([P, 2048], F32, name=f"pf{c}_{tg}",
                                     tag="pf")
                        nc.scalar.activation(
                            pf, gq, Exp, scale=2.0 / EPS,
                            bias=lnv[:, c : c + 1]
                        )
                        # alternate HWDGE rings so output stores overlap
                        deng = nc.sync if (tg // 2048) % 2 == 0 else nc.scalar
                        deng.dma_start(
                            out=out[c * P : (c + 1) * P, tg : tg + 2048],
                            in_=pf,
                        )
    if split_waits:
        _split_excess_waits(nc)
    return nc


_NC_CACHE = {}


def get_nc(N=8192, D=128, ncores=8):
    key = (N, D, ncores)
    if key not in _NC_CACHE:
        _NC_CACHE[key] = build_nc(N, D, ncores)
    return _NC_CACHE[key]


def make_in_maps(H_m, H_n, ncores=8):
    H_m = np.asarray(H_m, dtype=np.float32)
    H_n = np.asarray(H_n, dtype=np.float32)
    N = H_m.shape[0]
    R = N // ncores
    hmT = np.ascontiguousarray(H_m.T)
    return [
        {
            "hmT": hmT,
            "hnT": np.ascontiguousarray(H_n[c * R : (c + 1) * R].T),
        }
        for c in range(ncores)
    ]


def kernel(H_m, H_n):
    from concourse.bass_utils import run_bass_kernel_spmd

    ncores = 8
    nc = get_nc(N=np.asarray(H_m).shape[0], D=np.asarray(H_m).shape[1],
                ncores=ncores)
    in_maps = make_in_maps(H_m, H_n, ncores)
    res = run_bass_kernel_spmd(nc, in_maps, core_ids=list(range(ncores)))
    N = np.asarray(H_m).shape[0]
    R = N // ncores
    # each core returns its transposed column shard P^T[j_local, i]
    full = np.empty((N, N), dtype=np.float32)
    for c in range(ncores):
        full[:, c * R : (c + 1) * R] = res.results[c]["out"].T
    return full
